# revision 23
# baseline (speedup 1.0000x reference)
"""Bass/Trainium2 kernel for nn_CustomLoss_87952340287807.

Loss over B=8,388,608 Euler-angle triples:
    per-sample = 1 - |cross(vo, vt)| + dot(vo, vt),  summed.
vo/vt are unit vectors, so |cross| = sqrt(1 - dot^2) and only dot is needed.

dot = cosD*(u*U + v*V) + sinD*(u*V - v*U) + (cp*CP)*(cr*CR)
  u = sin(p~)cos(r~), v = sin(r~)   (o side; caps = t side)
  D = 2*pi*(yt - yo)
All trig from the Sin LUT (valid domain [-pi, pi]):
  sin(2pi(x-.5)) = Sin(2pi*x - pi)
  cos(2pi(x-.5)) = 1 - 2*h^2,  h = Sin(pi*x - pi/2)
  cosD = 1 - 2*hD^2, sinD = hD*(2 - 4*jD^2);  hD = Sin(pi*e), jD = Sin(pi*e/2)

Engine split: ScalarE does all LUT evals + the two r-side squares (Square
is a filler entry in every ACT table set, so no extra table load); DVE
runs the bilinear chain with three fused custom-DVE ops:
  COSPROD:  (1-2a^2)(1-2b^2)   -> cp*CP in one instruction
  COSMUL:   (1-2a^2)*t         -> cosD*a in one instruction
  SINMUL:   (s0-s1*a^2)*t      -> (2-4jD^2)*b in one instruction
One deferred Sqrt pass (separate ACT table) computes |cross| for the
whole core with a fused accumulator.

Sharding: pure data-parallel, batch split across 8 NeuronCores; each core
returns per-partition partial sums of dot and cross-norm; host reduces.
"""
import sys

import numpy as np

if "/opt/trn_rl_repo" not in sys.path:
    sys.path.insert(0, "/opt/trn_rl_repo")

import concourse.bacc as bacc
import concourse.mybir as mybir
import concourse.tile as tile
from concourse import dve_ops as dvo
from concourse.bass_utils import run_bass_kernel_spmd
from concourse.dve_spec import C0, C1, One, Spec, Src0, Src1, _has_src1, lower
from concourse.dve_spec import sq
from concourse.dve_uop import DveOpSpec

B = 8388608
NCORES = 8
S = B // NCORES          # 1,048,576 samples per core
P = 128
F = 2048                 # max samples per partition per segment
# ramp-up segments: small first chunks so compute starts ~5us after launch
# instead of waiting ~18us for a full 6 MiB tile pair
SEGS = [(0, 512), (512, 512)] + [(1024 + 1024 * k, 1024) for k in range(7)]
NSEG = len(SEGS)

AF = mybir.ActivationFunctionType
ALU = mybir.AluOpType
dt = mybir.dt
f32, f16 = dt.float32, dt.float16
PI = float(np.pi)

_cache = {}
last_results = None


def _reg(name, spec):
    """Register a custom DVE op at runtime (per-NEFF table, no firmware
    change). Computes the pinned uops sha the same way DveOp.compile does."""
    for op in dvo.OPS:
        if op.name == name:
            return op
    row = dvo._CUSTOM_DVE_ROW_BASE + len(dvo.OPS)
    assert row < 0x20, "custom-DVE opcode rows exhausted"
    ver = "v3"  # TRN2
    uops = lower(spec, ver=ver)
    sha = DveOpSpec(name=name, opcode=row, uops=uops,
                    rd1_en=_has_src1(spec)).sha(ver)
    op = dvo.DveOp(name, spec, subdim=False, uops_sha={ver: sha})
    dvo.OPS.append(op)
    dvo._SUB_OPCODE_FOR_NAME[name] = row
    dvo.CUSTOM_DVE_SPECS[name] = spec
    return op


# (1 - s0/2... s0=2: (1-2*Src0^2) * (1-2*Src1^2) = cosA*cosB from half-sines
COSPROD = _reg("COSPROD_ANT", Spec(
    body=(One - sq(Src0) * C0) * (One - sq(Src1) * C0)))
# (1-2*Src0^2) * Src1 = cosA * t from half-sine
COSMUL = _reg("COSMUL_ANT", Spec(
    body=(One - sq(Src0) * C0) * Src1))
# (s0 - s1*Src0^2) * Src1; s0=2, s1=4: (2-4*jD^2)*b = (sinD/hD)*b
SINMUL = _reg("SINMUL_ANT", Spec(
    body=(C0 - sq(Src0) * C1) * Src1))


def _build():
    nc = bacc.Bacc("TRN2", target_bir_lowering=False, debug=False)
    o_in = nc.declare_dram_parameter("out_in", [S, 3], f32, isOutput=False)
    t_in = nc.declare_dram_parameter("tgt_in", [S, 3], f32, isOutput=False)
    res = nc.declare_dram_parameter("res", [P, NSEG + 2], f32, isOutput=True)

    o_flat = o_in.ap().rearrange("(p n) c -> p (n c)", p=P)
    t_flat = t_in.ap().rearrange("(p n) c -> p (n c)", p=P)

    with tile.TileContext(nc) as tc:
        with tc.tile_pool(name="consts", bufs=1) as cpool, \
             tc.tile_pool(name="raw", bufs=2) as rawpool, \
             tc.tile_pool(name="sb", bufs=1) as pool, \
             tc.tile_pool(name="persist", bufs=1) as ppool:
            consts = {}
            for i, val in enumerate([-PI, -PI / 2, 1.0]):
                ct = cpool.tile([P, 1], f32, name=f"cst{i}", tag=f"cst{i}")
                nc.vector.memset(ct[:], val)
                consts[val] = ct[:]

            SP = S // P  # samples per partition (8192)
            q_all = ppool.tile([P, SP], f16, name="q_all", tag="q_all")
            dacc = ppool.tile([P, NSEG], f32, name="dacc", tag="dacc")
            cacc = ppool.tile([P, 2], f32, name="cacc", tag="cacc")

            def mk(tag, w, full=F, bufs=None):
                # full-width tag buffer, sliced to this segment's width so
                # ramp segments reuse the same SBUF
                t = pool.tile([P, full], f16, name=tag, tag=tag, bufs=bufs)
                return t[:, :w]

            def load(col0, fw):
                ro = rawpool.tile([P, 3 * F], f16, name="raw_o", tag="raw_o")
                nc.gpsimd.dma_start(ro[:, :3 * fw],
                                    o_flat[:, 3 * col0:3 * (col0 + fw)])
                rt = rawpool.tile([P, 3 * F], f16, name="raw_t", tag="raw_t")
                nc.gpsimd.dma_start(rt[:, :3 * fw],
                                    t_flat[:, 3 * col0:3 * (col0 + fw)])
                return ro, rt

            raws = load(*SEGS[0])
            for i, (col0, fw) in enumerate(SEGS):
                raw_o, raw_t = raws

                ov = raw_o[:, :3 * fw].rearrange("p (n c) -> p c n", c=3)
                tv = raw_t[:, :3 * fw].rearrange("p (n c) -> p c n", c=3)
                yo, yt = ov[:, 0, :], tv[:, 0, :]
                pr_o, pr_t = ov[:, 1:3, :], tv[:, 1:3, :]

                # full-angle sines [sp | sr] and half-angle sines [hp | hr]
                sf_o = mk("sf_o", 2 * fw, 2 * F, bufs=2)
                nc.scalar.activation(sf_o.rearrange("p (c n) -> p c n", c=2),
                                     pr_o, AF.Sin, bias=consts[-PI], scale=2 * PI)
                hh_o = mk("hh_o", 2 * fw, 2 * F, bufs=2)
                nc.scalar.activation(hh_o.rearrange("p (c n) -> p c n", c=2),
                                     pr_o, AF.Sin, bias=consts[-PI / 2], scale=PI)
                sf_t = mk("sf_t", 2 * fw, 2 * F, bufs=2)
                nc.scalar.activation(sf_t.rearrange("p (c n) -> p c n", c=2),
                                     pr_t, AF.Sin, bias=consts[-PI], scale=2 * PI)
                hh_t = mk("hh_t", 2 * fw, 2 * F, bufs=2)
                nc.scalar.activation(hh_t.rearrange("p (c n) -> p c n", c=2),
                                     pr_t, AF.Sin, bias=consts[-PI / 2], scale=PI)

                # r-side squares on ScalarE (Square is in every table set)
                qrr = mk("qrr", 2 * fw, 2 * F)
                nc.scalar.activation(qrr[:, :fw], hh_o[:, fw:2 * fw], AF.Square)
                nc.scalar.activation(qrr[:, fw:2 * fw], hh_t[:, fw:2 * fw],
                                     AF.Square)

                # yaw delta and its half/quarter sines
                e = mk("e", fw)
                nc.vector.tensor_sub(e, yt, yo)
                hD = mk("hD", fw)
                nc.scalar.activation(hD, e, AF.Sin, scale=PI)
                jD = mk("jD", fw)
                nc.scalar.activation(jD, e, AF.Sin, scale=PI / 2)

                # prefetch next segment's raws; emitted after this segment's
                # first consumers so their semaphore waits don't get coalesced
                # with the next segment's DMA completions
                if i + 1 < NSEG:
                    raws = load(*SEGS[i + 1])

                sp_o, sr_o = sf_o[:, :fw], sf_o[:, fw:2 * fw]
                sp_t, sr_t = sf_t[:, :fw], sf_t[:, fw:2 * fw]
                hp_o, hp_t = hh_o[:, :fw], hh_t[:, :fw]

                # consume sf/hh as early as possible so the next segment's ACT
                # writes into those buffers don't stall behind late DVE reads
                m2 = mk("m2", fw)
                nc.vector.tensor_mul(m2, sr_o, sr_t)
                x1 = mk("x1", fw)
                nc.vector._custom_dve(COSPROD, out=x1, in0=hp_o, in1=hp_t,
                                      s0=2.0)
                crr = mk("crr", 2 * fw, 2 * F)
                nc.vector.tensor_scalar(crr, qrr, -2.0, 1.0,
                                        ALU.mult, ALU.add)
                cr_o, cr_t = crr[:, :fw], crr[:, fw:2 * fw]
                u = mk("u", fw)
                nc.vector.tensor_mul(u, sp_o, cr_o)
                U_ = mk("U_", fw)
                nc.vector.tensor_mul(U_, sp_t, cr_t)
                m3 = mk("m3", fw)
                nc.vector.tensor_mul(m3, u, sr_t)
                m4 = mk("m4", fw)
                nc.vector.tensor_mul(m4, sr_o, U_)
                m1 = mk("m1", fw)
                nc.vector.tensor_mul(m1, u, U_)
                a = mk("a", fw)
                nc.vector.tensor_add(a, m1, m2)
                b = mk("b", fw)
                nc.vector.tensor_sub(b, m3, m4)
                x2 = mk("x2", fw)
                nc.vector.tensor_mul(x2, cr_o, cr_t)
                g = mk("m2", fw)
                nc.vector.tensor_mul(g, x1, x2)

                # rotation by D: p1 = cosD*a, q1 = sinD*b (fused)
                p1 = mk("u", fw)
                nc.vector._custom_dve(COSMUL, out=p1, in0=hD, in1=a, s0=2.0)
                t2b = mk("m3", fw)
                nc.vector._custom_dve(SINMUL, out=t2b, in0=jD, in1=b,
                                      s0=2.0, s1=4.0)
                q1 = mk("m4", fw)
                nc.vector.tensor_mul(q1, hD, t2b)
                s = mk("m1", fw)
                nc.vector.tensor_add(s, p1, q1)

                # dot = g + s; per-partition sum via a ScalarE copy-act whose
                # accumulator does the reduction (keeps DVE at 2x rate). The
                # last segment keeps the sum fused on DVE so the tail has no
                # cross-engine hop.
                dot = mk("a", fw)
                if i < NSEG - 3:
                    nc.vector.tensor_add(dot, g, s)
                    dsc = mk("x1", fw)
                    nc.scalar.activation(dsc, dot, AF.Copy,
                                         accum_out=dacc[:, i:i + 1])
                else:
                    nc.vector.scalar_tensor_tensor(
                        dot, g, 0.0, s, ALU.add, ALU.add,
                        accum_out=dacc[:, i:i + 1])

                # clamp and square -> q_all chunk
                hc = mk("m2", fw)
                nc.vector.tensor_scalar(hc, dot, -1.0, 1.0, ALU.max, ALU.min)
                nc.vector.tensor_mul(q_all[:, col0:col0 + fw], hc, hc)

            # deferred sqrt passes (sqrt table loads once): outputs land in
            # the by-then-dead raw pool buffers; the second pass covers the
            # last two segments so the serial tail is short
            m = 3 * F
            cnA = rawpool.tile([P, 3 * F], f16, name="cnA", tag="raw_o")
            nc.scalar.activation(cnA[:, :m], q_all[:, :m], AF.Sqrt,
                                 bias=consts[1.0], scale=-1.0,
                                 accum_out=cacc[:, 0:1])
            cnB = rawpool.tile([P, 3 * F], f16, name="cnB", tag="raw_t")
            nc.scalar.activation(cnB[:, :SP - m], q_all[:, m:], AF.Sqrt,
                                 bias=consts[1.0], scale=-1.0,
                                 accum_out=cacc[:, 1:2])

            nc.sync.dma_start(res[:, 0:NSEG], dacc[:])
            nc.sync.dma_start(res[:, NSEG:NSEG + 2], cacc[:])

    nc.compile()
    return nc


def kernel(output: np.ndarray, target: np.ndarray) -> np.ndarray:
    global last_results
    if "nc" not in _cache:
        _cache["nc"] = _build()
    nc = _cache["nc"]

    output = np.ascontiguousarray(output, dtype=np.float32)
    target = np.ascontiguousarray(target, dtype=np.float32)
    in_maps = [
        {"out_in": output[c * S:(c + 1) * S], "tgt_in": target[c * S:(c + 1) * S]}
        for c in range(NCORES)
    ]
    r = run_bass_kernel_spmd(nc, in_maps, list(range(NCORES)))
    last_results = r

    total = np.float64(B)
    for c in range(NCORES):
        out = r.results[c]["res"].astype(np.float64)
        total += out[:, 0:NSEG].sum() - out[:, NSEG:NSEG + 2].sum()
    return np.float32(total)


# revision 24
# speedup vs baseline: 1.0064x; 1.0064x over previous
"""Bass/Trainium2 kernel for nn_CustomLoss_87952340287807.

Loss over B=8,388,608 Euler-angle triples:
    per-sample = 1 - |cross(vo, vt)| + dot(vo, vt),  summed.
vo/vt are unit vectors, so |cross| = sqrt(1 - dot^2) and only dot is needed.

dot = cosD*(u*U + v*V) + sinD*(u*V - v*U) + (cp*CP)*(cr*CR)
  u = sin(p~)cos(r~), v = sin(r~)   (o side; caps = t side)
  D = 2*pi*(yt - yo)
All trig from the Sin LUT (valid domain [-pi, pi]):
  sin(2pi(x-.5)) = Sin(2pi*x - pi)
  cos(2pi(x-.5)) = 1 - 2*h^2,  h = Sin(pi*x - pi/2)
  cosD = 1 - 2*hD^2, sinD = hD*(2 - 4*jD^2);  hD = Sin(pi*e), jD = Sin(pi*e/2)

Engine split: ScalarE does all LUT evals + the two r-side squares (Square
is a filler entry in every ACT table set, so no extra table load); DVE
runs the bilinear chain with three fused custom-DVE ops:
  COSPROD:  (1-2a^2)(1-2b^2)   -> cp*CP in one instruction
  COSMUL:   (1-2a^2)*t         -> cosD*a in one instruction
  SINMUL:   (s0-s1*a^2)*t      -> (2-4jD^2)*b in one instruction
One deferred Sqrt pass (separate ACT table) computes |cross| for the
whole core with a fused accumulator.

Sharding: pure data-parallel, batch split across 8 NeuronCores; each core
returns per-partition partial sums of dot and cross-norm; host reduces.
"""
import sys

import numpy as np

if "/opt/trn_rl_repo" not in sys.path:
    sys.path.insert(0, "/opt/trn_rl_repo")

import concourse.bacc as bacc
import concourse.mybir as mybir
import concourse.tile as tile
from concourse import dve_ops as dvo
from concourse.bass_utils import run_bass_kernel_spmd
from concourse.dve_spec import C0, C1, One, Spec, Src0, Src1, _has_src1, lower
from concourse.dve_spec import sq
from concourse.dve_uop import DveOpSpec

B = 8388608
NCORES = 8
S = B // NCORES          # 1,048,576 samples per core
P = 128
F = 2048                 # max samples per partition per segment
# ramp-up segments: small first chunks so compute starts ~5us after launch
# instead of waiting ~18us for a full 6 MiB tile pair
SEGS = [(0, 512), (512, 512)] + [(1024 + 1024 * k, 1024) for k in range(7)]
NSEG = len(SEGS)

AF = mybir.ActivationFunctionType
ALU = mybir.AluOpType
dt = mybir.dt
f32, f16 = dt.float32, dt.float16
PI = float(np.pi)

_cache = {}
last_results = None


def _reg(name, spec):
    """Register a custom DVE op at runtime (per-NEFF table, no firmware
    change). Computes the pinned uops sha the same way DveOp.compile does."""
    for op in dvo.OPS:
        if op.name == name:
            return op
    row = dvo._CUSTOM_DVE_ROW_BASE + len(dvo.OPS)
    assert row < 0x20, "custom-DVE opcode rows exhausted"
    ver = "v3"  # TRN2
    uops = lower(spec, ver=ver)
    sha = DveOpSpec(name=name, opcode=row, uops=uops,
                    rd1_en=_has_src1(spec)).sha(ver)
    op = dvo.DveOp(name, spec, subdim=False, uops_sha={ver: sha})
    dvo.OPS.append(op)
    dvo._SUB_OPCODE_FOR_NAME[name] = row
    dvo.CUSTOM_DVE_SPECS[name] = spec
    return op


# (1 - s0/2... s0=2: (1-2*Src0^2) * (1-2*Src1^2) = cosA*cosB from half-sines
COSPROD = _reg("COSPROD_ANT", Spec(
    body=(One - sq(Src0) * C0) * (One - sq(Src1) * C0)))
# (1-2*Src0^2) * Src1 = cosA * t from half-sine
COSMUL = _reg("COSMUL_ANT", Spec(
    body=(One - sq(Src0) * C0) * Src1))
# (s0 - s1*Src0^2) * Src1; s0=2, s1=4: (2-4*jD^2)*b = (sinD/hD)*b
SINMUL = _reg("SINMUL_ANT", Spec(
    body=(C0 - sq(Src0) * C1) * Src1))


def _build():
    nc = bacc.Bacc("TRN2", target_bir_lowering=False, debug=False)
    o_in = nc.declare_dram_parameter("out_in", [S, 3], f32, isOutput=False)
    t_in = nc.declare_dram_parameter("tgt_in", [S, 3], f32, isOutput=False)
    res = nc.declare_dram_parameter("res", [P, NSEG + 2], f32, isOutput=True)

    o_flat = o_in.ap().rearrange("(p n) c -> p (n c)", p=P)
    t_flat = t_in.ap().rearrange("(p n) c -> p (n c)", p=P)

    with tile.TileContext(nc) as tc:
        with tc.tile_pool(name="consts", bufs=1) as cpool, \
             tc.tile_pool(name="raw", bufs=2) as rawpool, \
             tc.tile_pool(name="sb", bufs=1) as pool, \
             tc.tile_pool(name="persist", bufs=1) as ppool:
            consts = {}
            for i, val in enumerate([-PI, -PI / 2, 1.0]):
                ct = cpool.tile([P, 1], f32, name=f"cst{i}", tag=f"cst{i}")
                nc.vector.memset(ct[:], val)
                consts[val] = ct[:]

            SP = S // P  # samples per partition (8192)
            q_all = ppool.tile([P, SP], f16, name="q_all", tag="q_all")
            dacc = ppool.tile([P, NSEG], f32, name="dacc", tag="dacc")
            cacc = ppool.tile([P, 2], f32, name="cacc", tag="cacc")

            def mk(tag, w, full=F, bufs=None):
                # full-width tag buffer, sliced to this segment's width so
                # ramp segments reuse the same SBUF
                t = pool.tile([P, full], f16, name=tag, tag=tag, bufs=bufs)
                return t[:, :w]

            def load(col0, fw):
                ro = rawpool.tile([P, 3 * F], f16, name="raw_o", tag="raw_o")
                nc.gpsimd.dma_start(ro[:, :3 * fw],
                                    o_flat[:, 3 * col0:3 * (col0 + fw)])
                rt = rawpool.tile([P, 3 * F], f16, name="raw_t", tag="raw_t")
                nc.gpsimd.dma_start(rt[:, :3 * fw],
                                    t_flat[:, 3 * col0:3 * (col0 + fw)])
                return ro, rt

            raws = load(*SEGS[0])
            for i, (col0, fw) in enumerate(SEGS):
                raw_o, raw_t = raws

                ov = raw_o[:, :3 * fw].rearrange("p (n c) -> p c n", c=3)
                tv = raw_t[:, :3 * fw].rearrange("p (n c) -> p c n", c=3)
                yo, yt = ov[:, 0, :], tv[:, 0, :]
                pr_o, pr_t = ov[:, 1:3, :], tv[:, 1:3, :]

                # full-angle sines [sp | sr] and half-angle sines [hp | hr]
                sf_o = mk("sf_o", 2 * fw, 2 * F, bufs=2)
                nc.scalar.activation(sf_o.rearrange("p (c n) -> p c n", c=2),
                                     pr_o, AF.Sin, bias=consts[-PI], scale=2 * PI)
                hh_o = mk("hh_o", 2 * fw, 2 * F, bufs=2)
                nc.scalar.activation(hh_o.rearrange("p (c n) -> p c n", c=2),
                                     pr_o, AF.Sin, bias=consts[-PI / 2], scale=PI)
                sf_t = mk("sf_t", 2 * fw, 2 * F, bufs=2)
                nc.scalar.activation(sf_t.rearrange("p (c n) -> p c n", c=2),
                                     pr_t, AF.Sin, bias=consts[-PI], scale=2 * PI)
                hh_t = mk("hh_t", 2 * fw, 2 * F, bufs=2)
                nc.scalar.activation(hh_t.rearrange("p (c n) -> p c n", c=2),
                                     pr_t, AF.Sin, bias=consts[-PI / 2], scale=PI)

                # r-side squares on ScalarE (Square is in every table set)
                qrr = mk("qrr", 2 * fw, 2 * F)
                nc.scalar.activation(qrr[:, :fw], hh_o[:, fw:2 * fw], AF.Square)
                nc.scalar.activation(qrr[:, fw:2 * fw], hh_t[:, fw:2 * fw],
                                     AF.Square)

                # yaw delta and its half/quarter sines
                e = mk("e", fw)
                nc.vector.tensor_sub(e, yt, yo)
                hD = mk("hD", fw)
                nc.scalar.activation(hD, e, AF.Sin, scale=PI)
                jD = mk("jD", fw)
                nc.scalar.activation(jD, e, AF.Sin, scale=PI / 2)

                # prefetch next segment's raws; emitted after this segment's
                # first consumers so their semaphore waits don't get coalesced
                # with the next segment's DMA completions
                if i + 1 < NSEG:
                    raws = load(*SEGS[i + 1])

                sp_o, sr_o = sf_o[:, :fw], sf_o[:, fw:2 * fw]
                sp_t, sr_t = sf_t[:, :fw], sf_t[:, fw:2 * fw]
                hp_o, hp_t = hh_o[:, :fw], hh_t[:, :fw]

                # consume sf/hh as early as possible so the next segment's ACT
                # writes into those buffers don't stall behind late DVE reads
                m2 = mk("m2", fw)
                nc.vector.tensor_mul(m2, sr_o, sr_t)
                x1 = mk("x1", fw)
                nc.vector._custom_dve(COSPROD, out=x1, in0=hp_o, in1=hp_t,
                                      s0=2.0)
                crr = mk("crr", 2 * fw, 2 * F)
                nc.vector.tensor_scalar(crr, qrr, -2.0, 1.0,
                                        ALU.mult, ALU.add)
                cr_o, cr_t = crr[:, :fw], crr[:, fw:2 * fw]
                u = mk("u", fw)
                nc.vector.tensor_mul(u, sp_o, cr_o)
                U_ = mk("U_", fw)
                nc.vector.tensor_mul(U_, sp_t, cr_t)
                m3 = mk("m3", fw)
                nc.vector.tensor_mul(m3, u, sr_t)
                m4 = mk("m4", fw)
                nc.vector.tensor_mul(m4, sr_o, U_)
                m1 = mk("m1", fw)
                nc.vector.tensor_mul(m1, u, U_)
                a = mk("a", fw)
                nc.vector.tensor_add(a, m1, m2)
                b = mk("b", fw)
                nc.vector.tensor_sub(b, m3, m4)
                x2 = mk("x2", fw)
                nc.vector.tensor_mul(x2, cr_o, cr_t)
                g = mk("m2", fw)
                nc.vector.tensor_mul(g, x1, x2)

                # rotation by D: p1 = cosD*a, q1 = sinD*b (fused)
                p1 = mk("u", fw)
                nc.vector._custom_dve(COSMUL, out=p1, in0=hD, in1=a, s0=2.0)
                t2b = mk("m3", fw)
                nc.vector._custom_dve(SINMUL, out=t2b, in0=jD, in1=b,
                                      s0=2.0, s1=4.0)
                q1 = mk("m4", fw)
                nc.vector.tensor_mul(q1, hD, t2b)
                s = mk("m1", fw)
                nc.vector.tensor_add(s, p1, q1)

                # dot = g + s; per-partition sum via a ScalarE copy-act whose
                # accumulator does the reduction (keeps DVE at 2x rate). The
                # last segment keeps the sum fused on DVE so the tail has no
                # cross-engine hop.
                dot = mk("a", fw)
                if i + 1 < NSEG:
                    nc.vector.tensor_add(dot, g, s)
                    dsc = mk("x1", fw)
                    nc.scalar.activation(dsc, dot, AF.Copy,
                                         accum_out=dacc[:, i:i + 1])
                else:
                    nc.vector.scalar_tensor_tensor(
                        dot, g, 0.0, s, ALU.add, ALU.add,
                        accum_out=dacc[:, i:i + 1])

                # clamp and square -> q_all chunk
                hc = mk("m2", fw)
                nc.vector.tensor_scalar(hc, dot, -1.0, 1.0, ALU.max, ALU.min)
                nc.vector.tensor_mul(q_all[:, col0:col0 + fw], hc, hc)

            # deferred sqrt passes (sqrt table loads once): outputs land in
            # the by-then-dead raw pool buffers; the second pass covers the
            # last two segments so the serial tail is short
            m = 3 * F
            cnA = rawpool.tile([P, 3 * F], f16, name="cnA", tag="raw_o")
            nc.scalar.activation(cnA[:, :m], q_all[:, :m], AF.Sqrt,
                                 bias=consts[1.0], scale=-1.0,
                                 accum_out=cacc[:, 0:1])
            cnB = rawpool.tile([P, 3 * F], f16, name="cnB", tag="raw_t")
            nc.scalar.activation(cnB[:, :SP - m], q_all[:, m:], AF.Sqrt,
                                 bias=consts[1.0], scale=-1.0,
                                 accum_out=cacc[:, 1:2])

            nc.sync.dma_start(res[:, 0:NSEG], dacc[:])
            nc.sync.dma_start(res[:, NSEG:NSEG + 2], cacc[:])

    nc.compile()
    return nc


def kernel(output: np.ndarray, target: np.ndarray) -> np.ndarray:
    global last_results
    if "nc" not in _cache:
        _cache["nc"] = _build()
    nc = _cache["nc"]

    output = np.ascontiguousarray(output, dtype=np.float32)
    target = np.ascontiguousarray(target, dtype=np.float32)
    in_maps = [
        {"out_in": output[c * S:(c + 1) * S], "tgt_in": target[c * S:(c + 1) * S]}
        for c in range(NCORES)
    ]
    r = run_bass_kernel_spmd(nc, in_maps, list(range(NCORES)))
    last_results = r

    total = np.float64(B)
    for c in range(NCORES):
        out = r.results[c]["res"].astype(np.float64)
        total += out[:, 0:NSEG].sum() - out[:, NSEG:NSEG + 2].sum()
    return np.float32(total)
